# revision 25
# baseline (speedup 1.0000x reference)
"""Multi-head attention (b=4, n=4096, d_model=768, 16 heads x 128) on 8 TRN2
NeuronCores.

Sharding: core c handles batch c//2, head-group c%2 (8 heads = 1024 channels).
Host transposes q/k/v per batch to (768, n) and slices weights per head-group;
each core computes its partial output projection (n, 768); host sums the two
head-group partials per batch and adds the output bias.

Per-core dataflow (layouts chosen so no probability transpose is needed):
  phase 1a: qhT/khT = per-head projections producing (head_dim, n) via
           out = lhsT.T @ rhs with lhsT=W chunk, rhs=xT chunk; q/k bias added
           on ScalarE (Identity activation with per-partition bias AP).
  phase 1b: vh in natural (n, ch) layout, NO bias (v-bias folded in post-PV:
           out = pv/denom + bv since sum(probs)=denom).
  phase 2: per head: scoresT (keys, q) = khT_chunk.T @ qhT -> probsT computed
           by exp split across TWO engines: ScalarE exact Exp activation and
           VectorE fast-exp (Schraudolph: one tensor_scalar producing int16
           bits of the bf16 result, zero-mean-calibrated) in a 4:3 ratio
           matching the engines' measured rates. PV computed per q-block as
           out(q,129) = probsT_chunk.T @ [vh | 1] so column 128 accumulates
           the softmax denominator inside the same matmul; normalize on
           VectorE, transpose 128x128 blocks on TensorE into attnT (ch, n),
           v-bias added during the transposed copy.
  phase 3: out(n,768) = attnT_chunk.T @ Wo chunks, fp32 out.
"""

import numpy as np
import ml_dtypes
from contextlib import ExitStack

import concourse.bass as bass
import concourse.mybir as mybir
import concourse.tile as tile
from concourse import bacc
from concourse.bass_utils import run_bass_kernel_spmd
from concourse.masks import make_identity

B = 4
N_CTX = 4096
DM = 768
CH = 1024          # channels per core (8 heads x 128)
HD = 128
NH = 8             # heads per core
KC = DM // 128     # 6 contraction chunks for projections
NT = 512           # n-tile width
SCALE = HD ** -0.5
LOG2E = 1.4426950408889634
C_ZM = 7.219274    # zero-mean Schraudolph constant (2^7 mantissa domain)

f32 = mybir.dt.float32
bf16 = mybir.dt.bfloat16
i16 = mybir.dt.int16
fp8 = mybir.dt.float8e4
AF = mybir.ActivationFunctionType
ALU = mybir.AluOpType
PM = mybir.MatmulPerfMode
NTP = 1024         # phase-1 n-tile width (wide moving to amortize LDWEIGHTS)

_CACHE = {}


def _install_profhook():
    import contextlib, ctypes, sys, types

    if "antenv.axon_hooks" in sys.modules:
        return
    so = "/opt/axon/libaxon_pjrt.so"
    try:
        lib = ctypes.CDLL(so)
    except OSError:
        return
    if not hasattr(lib, "axon_start_nrt_profile"):
        return
    lib.axon_start_nrt_profile.argtypes = [ctypes.POINTER(ctypes.c_int64), ctypes.c_size_t]
    lib.axon_start_nrt_profile.restype = ctypes.c_int64
    lib.axon_stop_nrt_profile.argtypes = [ctypes.c_char_p]
    lib.axon_stop_nrt_profile.restype = ctypes.c_int64

    @contextlib.contextmanager
    def _hook(output_dir, device_ids):
        import jax
        jax.devices()
        if device_ids:
            ids = (ctypes.c_int64 * len(device_ids))(*device_ids)
            rc = lib.axon_start_nrt_profile(ids, len(device_ids))
        else:
            rc = lib.axon_start_nrt_profile(None, 0)
        if rc != 0:
            raise RuntimeError(f"axon_start_nrt_profile rc={rc}")
        try:
            yield
        finally:
            nf = lib.axon_stop_nrt_profile(str(output_dir).encode())
            print(f"profile: {nf} ntff file(s) in {output_dir}", file=sys.stderr)

    mod = types.ModuleType("antenv.axon_hooks")
    mod.get_axon_ntff_profile_hook = lambda: _hook
    mod.set_axon_ntff_profile_hook = lambda h: None
    sys.modules["antenv.axon_hooks"] = mod

    import concourse.bass_utils as bu
    bu.upload_artifacts = lambda tmpdir: "local://" + str(tmpdir)


def _build(n=N_CTX):
    nts = n // NT          # n tiles
    nbs = n // 128         # n blocks
    nc = bacc.Bacc(None, target_bir_lowering=False, debug=False, num_devices=8)

    qT = nc.declare_dram_parameter("qT", [DM, n], fp8, isOutput=False)
    kT = nc.declare_dram_parameter("kT", [DM, n], fp8, isOutput=False)
    vT = nc.declare_dram_parameter("vT", [DM, n], bf16, isOutput=False)
    wq = nc.declare_dram_parameter("wq", [DM, CH], fp8, isOutput=False)
    wk = nc.declare_dram_parameter("wk", [DM, CH], fp8, isOutput=False)
    wv = nc.declare_dram_parameter("wv", [DM, CH], bf16, isOutput=False)
    wo = nc.declare_dram_parameter("wo", [CH, DM], bf16, isOutput=False)
    bqk = nc.declare_dram_parameter("bqk", [128, 2 * NH], f32, isOutput=False)
    bvb = nc.declare_dram_parameter("bvb", [128, CH], f32, isOutput=False)
    out = nc.declare_dram_parameter("out", [n, DM], f32, isOutput=True)

    with tile.TileContext(nc) as tc, ExitStack() as ctx:
        dram = ctx.enter_context(tc.tile_pool(name="dram", bufs=1, space="DRAM"))
        qh_s = dram.tile([NH, 128, n], bf16)
        kh_s = dram.tile([NH, 128, n], bf16)
        vh_s = dram.tile([nbs, 128, NH, HD], bf16)

        singles = ctx.enter_context(tc.tile_pool(name="singles", bufs=1))

        bqk_t = singles.tile([128, 2 * NH], f32)
        nc.sync.dma_start(out=bqk_t[:, :], in_=bqk[:, :])
        bvb_t = singles.tile([128, CH], f32)
        nc.sync.dma_start(out=bvb_t[:, :], in_=bvb[:, :])

        # ---------------- phase 1a: q/k projections ----------------
        p1 = ExitStack()
        wpool = p1.enter_context(tc.tile_pool(name="w1", bufs=1))
        stream = p1.enter_context(tc.tile_pool(name="stream", bufs=3))
        stage1 = p1.enter_context(tc.tile_pool(name="stage1", bufs=4))
        pp = p1.enter_context(tc.tile_pool(name="pp", bufs=4, space="PSUM"))

        wq_t = wpool.tile([128, KC, CH], fp8, tag="wq")
        wk_t = wpool.tile([128, KC, CH], fp8, tag="wk")
        wv_t = wpool.tile([128, KC, CH], bf16, tag="wv")
        for w_t, w in ((wq_t, wq), (wk_t, wk), (wv_t, wv)):
            nc.sync.dma_start(
                out=w_t[:, :, :], in_=w[:].rearrange("(c p) m -> p c m", p=128)
            )

        ntsp = n // NTP
        for nt in range(ntsp):
            xs = []
            for name, x in (("q", qT), ("k", kT)):
                x_t = stream.tile([128, KC, NTP], fp8, tag=f"s{name}")
                nc.sync.dma_start(
                    out=x_t[:, :, :],
                    in_=x[:].rearrange("(c p) m -> p c m", p=128)[
                        :, :, nt * NTP : (nt + 1) * NTP
                    ],
                )
                xs.append(x_t)
            xv_t = stream.tile([128, KC, NTP], bf16, tag="sv")
            nc.sync.dma_start(
                out=xv_t[:, :, :],
                in_=vT[:].rearrange("(c p) m -> p c m", p=128)[
                    :, :, nt * NTP : (nt + 1) * NTP
                ],
            )
            for ti, (x_t, sc) in enumerate(((xs[0], qh_s), (xs[1], kh_s))):
                w_t = (wq_t, wk_t)[ti]
                for h in range(NH):
                    ps = pp.tile([128, NTP], f32, tag="proj")
                    # fp8 DoubleRow: contract chunk pair (2c, 2c+1); fp8
                    # moving operand caps at 1024 elems so go per 512-half
                    for half in range(NTP // 512):
                        hs = slice(half * 512, (half + 1) * 512)
                        for c in range(KC // 2):
                            nc.tensor.matmul(
                                ps[:, hs],
                                w_t[:, 2 * c : 2 * c + 2, h * HD : (h + 1) * HD],
                                x_t[:, 2 * c : 2 * c + 2, hs],
                                start=(c == 0),
                                stop=(c == KC // 2 - 1),
                                perf_mode=PM.DoubleRow,
                                skip_group_check=True,
                            )
                    st = stage1.tile([128, NTP], bf16, tag="qkst")
                    # bias add on ScalarE (idle during phase 1)
                    nc.scalar.activation(
                        st[:, :], ps[:, :], AF.Identity,
                        bias=bqk_t[:, ti * NH + h : ti * NH + h + 1],
                    )
                    nc.sync.dma_start(
                        out=sc[h, :, nt * NTP : (nt + 1) * NTP], in_=st[:, :]
                    )
            # v projection for this n-tile (no bias; folded post-PV)
            for nbq in range(NTP // 128):
                ps = pp.tile([128, CH], f32, tag="proj")
                for ct in range(CH // 512):
                    cs = slice(ct * 512, (ct + 1) * 512)
                    for c in range(KC):
                        nc.tensor.matmul(
                            ps[:, cs],
                            xv_t[:, c, nbq * 128 : (nbq + 1) * 128],
                            wv_t[:, c, cs],
                            start=(c == 0),
                            stop=(c == KC - 1),
                            skip_group_check=True,
                        )
                st = stage1.tile([128, CH], bf16, tag="vst")
                nc.vector.tensor_copy(st[:, :], ps[:, :])
                nc.sync.dma_start(
                    out=vh_s[nt * (NTP // 128) + nbq, :, :, :], in_=st[:, :],
                )

        p1.close()

        # ---------------- phase 2: attention per head ----------------
        atile = ctx.enter_context(tc.tile_pool(name="atile", bufs=NH))
        p2 = ExitStack()
        qk2 = p2.enter_context(tc.tile_pool(name="qk2", bufs=2))
        vh2 = p2.enter_context(tc.tile_pool(name="vh2", bufs=2))
        probs = p2.enter_context(tc.tile_pool(name="probs", bufs=7))
        stage2 = p2.enter_context(tc.tile_pool(name="stage2", bufs=4))
        small = p2.enter_context(tc.tile_pool(name="small", bufs=4))
        scp = p2.enter_context(tc.tile_pool(name="scp", bufs=2, space="PSUM"))
        pvp = p2.enter_context(tc.tile_pool(name="pvp", bufs=4, space="PSUM"))

        GG = 12  # key-chunks per buffered probs tile
        EG = 2   # key-chunks per exp op (2 psum banks)
        # Schraudolph constants: probs = exp(score*SCALE) as bf16 bits
        sch_s1 = SCALE * LOG2E * 128.0
        sch_s2 = 127.0 * 128.0 - C_ZM
        expctr = 0  # global exp-group counter for the 4:3 scalar:vector split
        at_ts = []
        for h in range(NH):
            qh_t = qk2.tile([128, n], bf16, tag="qh")
            nc.sync.dma_start(out=qh_t[:, :], in_=qh_s[h, :, :])
            kh_t = qk2.tile([128, n], bf16, tag="kh")
            nc.sync.dma_start(out=kh_t[:, :], in_=kh_s[h, :, :])
            vh_t = vh2.tile([128, nbs, HD + 1], bf16, tag="vh")
            nc.sync.dma_start(
                out=vh_t[:, :, 0:HD],
                in_=vh_s[:, :, h, :].rearrange("c p j -> p c j"),
            )
            nc.vector.memset(vh_t[:, :, HD], 1.0)

            at_t = atile.tile([128, n], bf16, tag="at")
            at_ts.append(at_t)

            prev_pq = None
            for qt in range(nts + 1):
              if qt < nts:
                pq_list = []
                for gq in range((nbs + GG - 1) // GG):
                    kb0 = gq * GG
                    kbn = min(GG, nbs - kb0)
                    pq = probs.tile([128, GG, NT], bf16, tag="pq")
                    pq_list.append(pq)
                    off = 0
                    while off < kbn:
                        gsz = min(EG, kbn - off)
                        sc_t = scp.tile([128, EG, NT], f32, tag="sc")
                        for i in range(gsz):
                            kb = kb0 + off + i
                            nc.tensor.matmul(
                                sc_t[:, i, :],
                                kh_t[:, kb * 128 : (kb + 1) * 128],
                                qh_t[:, qt * NT : (qt + 1) * NT],
                                start=True,
                                stop=True,
                            )
                        if expctr % 13 < 7:
                            nc.scalar.activation(
                                pq[:, off : off + gsz, :], sc_t[:, 0:gsz, :],
                                AF.Exp, scale=SCALE,
                            )
                        else:
                            nc.vector.tensor_scalar(
                                pq[:, off : off + gsz, :].bitcast(i16),
                                sc_t[:, 0:gsz, :],
                                sch_s1, sch_s2, ALU.mult, ALU.add,
                            )
                        expctr += 1
                        off += gsz
              if qt > 0:
                plist = prev_pq
                for qb in range(4):
                    pv = pvp.tile([128, HD + 1], f32, tag="pv")
                    for kb in range(nbs):
                        nc.tensor.matmul(
                            pv[:, :],
                            plist[kb // GG][:, kb % GG, qb * 128 : (qb + 1) * 128],
                            vh_t[:, kb, :],
                            start=(kb == 0),
                            stop=(kb == nbs - 1),
                            skip_group_check=True,
                        )
                    rec = small.tile([128, 1], f32, tag="rec")
                    nc.vector.reciprocal(rec[:, :], pv[:, HD : HD + 1])
                    st = stage2.tile([128, HD], bf16, tag="nst")
                    # st = pv/denom + bv  (one fused DVE op)
                    nc.vector.scalar_tensor_tensor(
                        st[:, :], pv[:, 0:HD], rec[:, :],
                        bvb_t[:, h * HD : (h + 1) * HD],
                        ALU.mult, ALU.add,
                    )
                    qb_g = (qt - 1) * 4 + qb
                    # transpose via DMA xbar (frees TensorE + avoids queue stall)
                    nc.sync.dma_start_transpose(
                        out=at_t[:, qb_g * 128 : (qb_g + 1) * 128], in_=st[:, :]
                    )
              if qt < nts:
                prev_pq = pq_list

        p2.close()

        # ---------------- phase 3: output projection ----------------
        wpool3 = ctx.enter_context(tc.tile_pool(name="w3", bufs=1))
        stage3 = ctx.enter_context(tc.tile_pool(name="stage3", bufs=4))
        opp = ctx.enter_context(tc.tile_pool(name="opp", bufs=2, space="PSUM"))
        wo_t = wpool3.tile([128, NH, DM], bf16, tag="wo")
        nc.sync.dma_start(
            out=wo_t[:, :, :], in_=wo[:].rearrange("(c p) m -> p c m", p=128)
        )
        for nb in range(nbs):
            po = opp.tile([128, DM], f32, tag="po")
            for h in range(NH):
                lhsT = at_ts[h][:, nb * 128 : (nb + 1) * 128]
                nc.tensor.matmul(
                    po[:, 0:512], lhsT, wo_t[:, h, 0:512],
                    start=(h == 0), stop=(h == NH - 1), skip_group_check=True,
                )
                nc.tensor.matmul(
                    po[:, 512:DM], lhsT, wo_t[:, h, 512:DM],
                    start=(h == 0), stop=(h == NH - 1), skip_group_check=True,
                )
            so = stage3.tile([128, DM], f32, tag="ost")
            nc.vector.tensor_copy(so[:, :], po[:, :])
            nc.sync.dma_start(out=out[nb * 128 : (nb + 1) * 128, :], in_=so[:, :])

    nc.compile()
    return nc


def _get_nc(n=N_CTX):
    if n not in _CACHE:
        _CACHE[n] = _build(n)
    return _CACHE[n]


def _shard_inputs(q, k, v, Wq, bq, Wk, bk, Wv, bv, Wo, bo):
    bf = ml_dtypes.bfloat16
    in_maps = []
    for c in range(8):
        bi, hg = c // 2, c % 2
        s = slice(hg * CH, (hg + 1) * CH)
        bqk_c = np.empty((128, 2 * NH), np.float32)
        for h in range(NH):
            bqk_c[:, h] = bq[hg * CH + h * HD : hg * CH + (h + 1) * HD]
            bqk_c[:, NH + h] = bk[hg * CH + h * HD : hg * CH + (h + 1) * HD]
        bvb_c = np.ascontiguousarray(
            np.broadcast_to(bv[s].astype(np.float32), (128, CH))
        )
        f8 = ml_dtypes.float8_e4m3fn
        in_maps.append({
            "qT": np.ascontiguousarray(q[bi].T).astype(f8),
            "kT": np.ascontiguousarray(k[bi].T).astype(f8),
            "vT": np.ascontiguousarray(v[bi].T).astype(bf),
            "wq": np.ascontiguousarray(Wq[:, s]).astype(f8),
            "wk": np.ascontiguousarray(Wk[:, s]).astype(f8),
            "wv": np.ascontiguousarray(Wv[:, s]).astype(bf),
            "wo": np.ascontiguousarray(Wo[s, :]).astype(bf),
            "bqk": bqk_c,
            "bvb": bvb_c,
        })
    return in_maps


def kernel(q, k, v, Wq, bq, Wk, bk, Wv, bv, Wo, bo, _profile=False):
    import os

    q = np.asarray(q); k = np.asarray(k); v = np.asarray(v)
    n = q.shape[1]
    nc = _get_nc(n)
    in_maps = _shard_inputs(
        q, k, v, np.asarray(Wq), np.asarray(bq), np.asarray(Wk), np.asarray(bk),
        np.asarray(Wv), np.asarray(bv), np.asarray(Wo), np.asarray(bo),
    )
    profile = _profile or bool(int(os.environ.get("KERNEL_PROFILE", "0")))
    if profile:
        _install_profhook()
    res = run_bass_kernel_spmd(nc, in_maps, list(range(8)), trace=profile)
    if profile and res.exec_time_ns is not None:
        print(f"HW exec time: {res.exec_time_ns} ns")
    bo32 = np.asarray(bo, np.float32)
    out = np.empty((q.shape[0], n, DM), np.float32)
    for bi in range(q.shape[0]):
        out[bi] = res.results[2 * bi]["out"] + res.results[2 * bi + 1]["out"] + bo32
    return out


# revision 26
# speedup vs baseline: 1.2227x; 1.2227x over previous
"""Multi-head attention (b=4, n=4096, d_model=768, 16 heads x 128) on 8 TRN2
NeuronCores.

Sharding: core c handles batch c//2, head-group c%2 (8 heads = 1024 channels).
Host transposes q/k/v per batch to (768, n) and slices weights per head-group;
each core computes its partial output projection (n, 768); host sums the two
head-group partials per batch and adds the output bias.

Per-core dataflow (layouts chosen so no probability transpose is needed):
  phase 1a: qhT/khT = per-head projections producing (head_dim, n) via
           out = lhsT.T @ rhs with lhsT=W chunk, rhs=xT chunk; q/k bias added
           on ScalarE (Identity activation with per-partition bias AP).
  phase 1b: vh in natural (n, ch) layout, NO bias (v-bias folded in post-PV:
           out = pv/denom + bv since sum(probs)=denom).
  phase 2: per head: scoresT (keys, q) = khT_chunk.T @ qhT -> probsT computed
           by exp split across TWO engines: ScalarE exact Exp activation and
           VectorE fast-exp (Schraudolph: one tensor_scalar producing int16
           bits of the bf16 result, zero-mean-calibrated) in a 4:3 ratio
           matching the engines' measured rates. PV computed per q-block as
           out(q,129) = probsT_chunk.T @ [vh | 1] so column 128 accumulates
           the softmax denominator inside the same matmul; normalize on
           VectorE, transpose 128x128 blocks on TensorE into attnT (ch, n),
           v-bias added during the transposed copy.
  phase 3: out(n,768) = attnT_chunk.T @ Wo chunks, fp32 out.
"""

import numpy as np
import ml_dtypes
from contextlib import ExitStack

import concourse.bass as bass
import concourse.mybir as mybir
import concourse.tile as tile
from concourse import bacc
from concourse.bass_utils import run_bass_kernel_spmd
from concourse.masks import make_identity

B = 4
N_CTX = 4096
DM = 768
CH = 1024          # channels per core (8 heads x 128)
HD = 128
NH = 8             # heads per core
KC = DM // 128     # 6 contraction chunks for projections
NT = 512           # n-tile width
SCALE = HD ** -0.5
LOG2E = 1.4426950408889634
C_ZM = 7.219274    # zero-mean Schraudolph constant (2^7 mantissa domain)

f32 = mybir.dt.float32
bf16 = mybir.dt.bfloat16
i16 = mybir.dt.int16
fp8 = mybir.dt.float8e4
AF = mybir.ActivationFunctionType
ALU = mybir.AluOpType
PM = mybir.MatmulPerfMode
NTP = 1024         # phase-1 n-tile width (wide moving to amortize LDWEIGHTS)

_CACHE = {}


def _install_profhook():
    import contextlib, ctypes, sys, types

    if "antenv.axon_hooks" in sys.modules:
        return
    so = "/opt/axon/libaxon_pjrt.so"
    try:
        lib = ctypes.CDLL(so)
    except OSError:
        return
    if not hasattr(lib, "axon_start_nrt_profile"):
        return
    lib.axon_start_nrt_profile.argtypes = [ctypes.POINTER(ctypes.c_int64), ctypes.c_size_t]
    lib.axon_start_nrt_profile.restype = ctypes.c_int64
    lib.axon_stop_nrt_profile.argtypes = [ctypes.c_char_p]
    lib.axon_stop_nrt_profile.restype = ctypes.c_int64

    @contextlib.contextmanager
    def _hook(output_dir, device_ids):
        import jax
        jax.devices()
        if device_ids:
            ids = (ctypes.c_int64 * len(device_ids))(*device_ids)
            rc = lib.axon_start_nrt_profile(ids, len(device_ids))
        else:
            rc = lib.axon_start_nrt_profile(None, 0)
        if rc != 0:
            raise RuntimeError(f"axon_start_nrt_profile rc={rc}")
        try:
            yield
        finally:
            nf = lib.axon_stop_nrt_profile(str(output_dir).encode())
            print(f"profile: {nf} ntff file(s) in {output_dir}", file=sys.stderr)

    mod = types.ModuleType("antenv.axon_hooks")
    mod.get_axon_ntff_profile_hook = lambda: _hook
    mod.set_axon_ntff_profile_hook = lambda h: None
    sys.modules["antenv.axon_hooks"] = mod

    import concourse.bass_utils as bu
    bu.upload_artifacts = lambda tmpdir: "local://" + str(tmpdir)


def _build(n=N_CTX):
    nts = n // NT          # n tiles
    nbs = n // 128         # n blocks
    nc = bacc.Bacc(None, target_bir_lowering=False, debug=False, num_devices=8)

    qT = nc.declare_dram_parameter("qT", [DM, n], fp8, isOutput=False)
    kT = nc.declare_dram_parameter("kT", [DM, n], fp8, isOutput=False)
    vT = nc.declare_dram_parameter("vT", [DM, n], bf16, isOutput=False)
    wq = nc.declare_dram_parameter("wq", [DM, CH], fp8, isOutput=False)
    wk = nc.declare_dram_parameter("wk", [DM, CH], fp8, isOutput=False)
    wv = nc.declare_dram_parameter("wv", [DM, CH], bf16, isOutput=False)
    wo = nc.declare_dram_parameter("wo", [CH, DM], bf16, isOutput=False)
    bqk = nc.declare_dram_parameter("bqk", [128, 2 * NH], f32, isOutput=False)
    bvb = nc.declare_dram_parameter("bvb", [128, CH], f32, isOutput=False)
    out = nc.declare_dram_parameter("out", [n, DM], f32, isOutput=True)

    with tile.TileContext(nc) as tc, ExitStack() as ctx:
        dram = ctx.enter_context(tc.tile_pool(name="dram", bufs=1, space="DRAM"))
        qh_s = dram.tile([NH, 128, n], bf16)
        kh_s = dram.tile([NH, 128, n], bf16)
        vh_s = dram.tile([nbs, 128, NH, HD], bf16)

        singles = ctx.enter_context(tc.tile_pool(name="singles", bufs=1))

        bqk_t = singles.tile([128, 2 * NH], f32)
        nc.sync.dma_start(out=bqk_t[:, :], in_=bqk[:, :])
        bvb_t = singles.tile([128, CH], f32)
        nc.sync.dma_start(out=bvb_t[:, :], in_=bvb[:, :])

        # ---------------- phase 1a: q/k projections ----------------
        p1 = ExitStack()
        wpool = p1.enter_context(tc.tile_pool(name="w1", bufs=1))
        stream = p1.enter_context(tc.tile_pool(name="stream", bufs=3))
        stage1 = p1.enter_context(tc.tile_pool(name="stage1", bufs=4))
        pp = p1.enter_context(tc.tile_pool(name="pp", bufs=4, space="PSUM"))

        wq_t = wpool.tile([128, KC, CH], fp8, tag="wq")
        wk_t = wpool.tile([128, KC, CH], fp8, tag="wk")
        wv_t = wpool.tile([128, KC, CH], bf16, tag="wv")
        for w_t, w in ((wq_t, wq), (wk_t, wk), (wv_t, wv)):
            nc.sync.dma_start(
                out=w_t[:, :, :], in_=w[:].rearrange("(c p) m -> p c m", p=128)
            )

        ntsp = n // NTP
        for nt in range(ntsp):
            xs = []
            for name, x in (("q", qT), ("k", kT)):
                x_t = stream.tile([128, KC, NTP], fp8, tag=f"s{name}")
                nc.sync.dma_start(
                    out=x_t[:, :, :],
                    in_=x[:].rearrange("(c p) m -> p c m", p=128)[
                        :, :, nt * NTP : (nt + 1) * NTP
                    ],
                )
                xs.append(x_t)
            xv_t = stream.tile([128, KC, NTP], bf16, tag="sv")
            nc.sync.dma_start(
                out=xv_t[:, :, :],
                in_=vT[:].rearrange("(c p) m -> p c m", p=128)[
                    :, :, nt * NTP : (nt + 1) * NTP
                ],
            )
            for ti, (x_t, sc) in enumerate(((xs[0], qh_s), (xs[1], kh_s))):
                w_t = (wq_t, wk_t)[ti]
                for h in range(NH):
                    ps = pp.tile([128, NTP], f32, tag="proj")
                    # fp8 DoubleRow: contract chunk pair (2c, 2c+1); fp8
                    # moving operand caps at 1024 elems so go per 512-half
                    for half in range(NTP // 512):
                        hs = slice(half * 512, (half + 1) * 512)
                        for c in range(KC // 2):
                            nc.tensor.matmul(
                                ps[:, hs],
                                w_t[:, 2 * c : 2 * c + 2, h * HD : (h + 1) * HD],
                                x_t[:, 2 * c : 2 * c + 2, hs],
                                start=(c == 0),
                                stop=(c == KC // 2 - 1),
                                perf_mode=PM.DoubleRow,
                                skip_group_check=True,
                            )
                    st = stage1.tile([128, NTP], bf16, tag="qkst")
                    # bias add on ScalarE (idle during phase 1)
                    nc.scalar.activation(
                        st[:, :], ps[:, :], AF.Identity,
                        bias=bqk_t[:, ti * NH + h : ti * NH + h + 1],
                    )
                    nc.sync.dma_start(
                        out=sc[h, :, nt * NTP : (nt + 1) * NTP], in_=st[:, :]
                    )
            # v projection for this n-tile (no bias; folded post-PV)
            for nbq in range(NTP // 128):
                ps = pp.tile([128, CH], f32, tag="proj")
                for ct in range(CH // 512):
                    cs = slice(ct * 512, (ct + 1) * 512)
                    for c in range(KC):
                        nc.tensor.matmul(
                            ps[:, cs],
                            xv_t[:, c, nbq * 128 : (nbq + 1) * 128],
                            wv_t[:, c, cs],
                            start=(c == 0),
                            stop=(c == KC - 1),
                            skip_group_check=True,
                        )
                st = stage1.tile([128, CH], bf16, tag="vst")
                nc.vector.tensor_copy(st[:, :], ps[:, :])
                nc.sync.dma_start(
                    out=vh_s[nt * (NTP // 128) + nbq, :, :, :], in_=st[:, :],
                )

        p1.close()

        # ---------------- phase 2: attention per head ----------------
        atile = ctx.enter_context(tc.tile_pool(name="atile", bufs=NH))
        p2 = ExitStack()
        qk2 = p2.enter_context(tc.tile_pool(name="qk2", bufs=2))
        vh2 = p2.enter_context(tc.tile_pool(name="vh2", bufs=2))
        probs = p2.enter_context(tc.tile_pool(name="probs", bufs=7))
        stage2 = p2.enter_context(tc.tile_pool(name="stage2", bufs=4))
        small = p2.enter_context(tc.tile_pool(name="small", bufs=4))
        scp = p2.enter_context(tc.tile_pool(name="scp", bufs=2, space="PSUM"))
        pvp = p2.enter_context(tc.tile_pool(name="pvp", bufs=2, space="PSUM"))

        GG = 12  # key-chunks per buffered probs tile
        EG = 3   # key-chunks per exp op (3 psum banks)
        # Schraudolph constants: probs = exp(score*SCALE) as bf16 bits
        sch_s1 = SCALE * LOG2E * 128.0
        sch_s2 = 127.0 * 128.0 - C_ZM
        expctr = 0  # global exp-group counter for the 4:3 scalar:vector split
        at_ts = []
        for h in range(NH):
            qh_t = qk2.tile([128, n], bf16, tag="qh")
            nc.sync.dma_start(out=qh_t[:, :], in_=qh_s[h, :, :])
            kh_t = qk2.tile([128, n], bf16, tag="kh")
            nc.sync.dma_start(out=kh_t[:, :], in_=kh_s[h, :, :])
            vh_t = vh2.tile([128, nbs, HD + 1], bf16, tag="vh")
            nc.sync.dma_start(
                out=vh_t[:, :, 0:HD],
                in_=vh_s[:, :, h, :].rearrange("c p j -> p c j"),
            )
            nc.vector.memset(vh_t[:, :, HD], 1.0)

            at_t = atile.tile([128, n], bf16, tag="at")
            at_ts.append(at_t)

            prev_pq = None
            for qt in range(nts + 1):
              if qt < nts:
                pq_list = []
                for gq in range((nbs + GG - 1) // GG):
                    kb0 = gq * GG
                    kbn = min(GG, nbs - kb0)
                    pq = probs.tile([128, GG, NT], bf16, tag="pq")
                    pq_list.append(pq)
                    off = 0
                    while off < kbn:
                        gsz = min(EG, kbn - off)
                        sc_t = scp.tile([128, EG, NT], f32, tag="sc")
                        for i in range(gsz):
                            kb = kb0 + off + i
                            nc.tensor.matmul(
                                sc_t[:, i, :],
                                kh_t[:, kb * 128 : (kb + 1) * 128],
                                qh_t[:, qt * NT : (qt + 1) * NT],
                                start=True,
                                stop=True,
                            )
                        if expctr % 7 < 4:
                            nc.scalar.activation(
                                pq[:, off : off + gsz, :], sc_t[:, 0:gsz, :],
                                AF.Exp, scale=SCALE,
                            )
                        else:
                            nc.vector.tensor_scalar(
                                pq[:, off : off + gsz, :].bitcast(i16),
                                sc_t[:, 0:gsz, :],
                                sch_s1, sch_s2, ALU.mult, ALU.add,
                            )
                        expctr += 1
                        off += gsz
              if qt > 0:
                plist = prev_pq
                for qb in range(4):
                    pv = pvp.tile([128, HD + 1], f32, tag="pv")
                    for kb in range(nbs):
                        nc.tensor.matmul(
                            pv[:, :],
                            plist[kb // GG][:, kb % GG, qb * 128 : (qb + 1) * 128],
                            vh_t[:, kb, :],
                            start=(kb == 0),
                            stop=(kb == nbs - 1),
                            skip_group_check=True,
                        )
                    rec = small.tile([128, 1], f32, tag="rec")
                    nc.vector.reciprocal(rec[:, :], pv[:, HD : HD + 1])
                    st = stage2.tile([128, HD], bf16, tag="nst")
                    # st = pv/denom + bv  (one fused DVE op)
                    nc.vector.scalar_tensor_tensor(
                        st[:, :], pv[:, 0:HD], rec[:, :],
                        bvb_t[:, h * HD : (h + 1) * HD],
                        ALU.mult, ALU.add,
                    )
                    qb_g = (qt - 1) * 4 + qb
                    # transpose via DMA xbar (frees TensorE + avoids queue stall)
                    nc.sync.dma_start_transpose(
                        out=at_t[:, qb_g * 128 : (qb_g + 1) * 128], in_=st[:, :]
                    )
              if qt < nts:
                prev_pq = pq_list

        p2.close()

        # ---------------- phase 3: output projection ----------------
        wpool3 = ctx.enter_context(tc.tile_pool(name="w3", bufs=1))
        stage3 = ctx.enter_context(tc.tile_pool(name="stage3", bufs=4))
        opp = ctx.enter_context(tc.tile_pool(name="opp", bufs=2, space="PSUM"))
        wo_t = wpool3.tile([128, NH, DM], bf16, tag="wo")
        nc.sync.dma_start(
            out=wo_t[:, :, :], in_=wo[:].rearrange("(c p) m -> p c m", p=128)
        )
        for nb in range(nbs):
            po = opp.tile([128, DM], f32, tag="po")
            for h in range(NH):
                lhsT = at_ts[h][:, nb * 128 : (nb + 1) * 128]
                nc.tensor.matmul(
                    po[:, 0:512], lhsT, wo_t[:, h, 0:512],
                    start=(h == 0), stop=(h == NH - 1), skip_group_check=True,
                )
                nc.tensor.matmul(
                    po[:, 512:DM], lhsT, wo_t[:, h, 512:DM],
                    start=(h == 0), stop=(h == NH - 1), skip_group_check=True,
                )
            so = stage3.tile([128, DM], f32, tag="ost")
            nc.vector.tensor_copy(so[:, :], po[:, :])
            nc.sync.dma_start(out=out[nb * 128 : (nb + 1) * 128, :], in_=so[:, :])

    nc.compile()
    return nc


def _get_nc(n=N_CTX):
    if n not in _CACHE:
        _CACHE[n] = _build(n)
    return _CACHE[n]


def _shard_inputs(q, k, v, Wq, bq, Wk, bk, Wv, bv, Wo, bo):
    bf = ml_dtypes.bfloat16
    in_maps = []
    for c in range(8):
        bi, hg = c // 2, c % 2
        s = slice(hg * CH, (hg + 1) * CH)
        bqk_c = np.empty((128, 2 * NH), np.float32)
        for h in range(NH):
            bqk_c[:, h] = bq[hg * CH + h * HD : hg * CH + (h + 1) * HD]
            bqk_c[:, NH + h] = bk[hg * CH + h * HD : hg * CH + (h + 1) * HD]
        bvb_c = np.ascontiguousarray(
            np.broadcast_to(bv[s].astype(np.float32), (128, CH))
        )
        f8 = ml_dtypes.float8_e4m3fn
        in_maps.append({
            "qT": np.ascontiguousarray(q[bi].T).astype(f8),
            "kT": np.ascontiguousarray(k[bi].T).astype(f8),
            "vT": np.ascontiguousarray(v[bi].T).astype(bf),
            "wq": np.ascontiguousarray(Wq[:, s]).astype(f8),
            "wk": np.ascontiguousarray(Wk[:, s]).astype(f8),
            "wv": np.ascontiguousarray(Wv[:, s]).astype(bf),
            "wo": np.ascontiguousarray(Wo[s, :]).astype(bf),
            "bqk": bqk_c,
            "bvb": bvb_c,
        })
    return in_maps


def kernel(q, k, v, Wq, bq, Wk, bk, Wv, bv, Wo, bo, _profile=False):
    import os

    q = np.asarray(q); k = np.asarray(k); v = np.asarray(v)
    n = q.shape[1]
    nc = _get_nc(n)
    in_maps = _shard_inputs(
        q, k, v, np.asarray(Wq), np.asarray(bq), np.asarray(Wk), np.asarray(bk),
        np.asarray(Wv), np.asarray(bv), np.asarray(Wo), np.asarray(bo),
    )
    profile = _profile or bool(int(os.environ.get("KERNEL_PROFILE", "0")))
    if profile:
        _install_profhook()
    res = run_bass_kernel_spmd(nc, in_maps, list(range(8)), trace=profile)
    if profile and res.exec_time_ns is not None:
        print(f"HW exec time: {res.exec_time_ns} ns")
    bo32 = np.asarray(bo, np.float32)
    out = np.empty((q.shape[0], n, DM), np.float32)
    for bi in range(q.shape[0]):
        out[bi] = res.results[2 * bi]["out"] + res.results[2 * bi + 1]["out"] + bo32
    return out


# revision 27
# speedup vs baseline: 1.2276x; 1.0041x over previous
"""Multi-head attention (b=4, n=4096, d_model=768, 16 heads x 128) on 8 TRN2
NeuronCores.

Sharding: core c handles batch c//2, head-group c%2 (8 heads = 1024 channels).
Host transposes q/k/v per batch to (768, n) and slices weights per head-group;
each core computes its partial output projection (n, 768); host sums the two
head-group partials per batch and adds the output bias.

Per-core dataflow (layouts chosen so no probability transpose is needed):
  phase 1a: qhT/khT = per-head projections producing (head_dim, n) via
           out = lhsT.T @ rhs with lhsT=W chunk, rhs=xT chunk; q/k bias added
           on ScalarE (Identity activation with per-partition bias AP).
  phase 1b: vh in natural (n, ch) layout, NO bias (v-bias folded in post-PV:
           out = pv/denom + bv since sum(probs)=denom).
  phase 2: per head: scoresT (keys, q) = khT_chunk.T @ qhT -> probsT computed
           by exp split across TWO engines: ScalarE exact Exp activation and
           VectorE fast-exp (Schraudolph: one tensor_scalar producing int16
           bits of the bf16 result, zero-mean-calibrated) in a 4:3 ratio
           matching the engines' measured rates. PV computed per q-block as
           out(q,129) = probsT_chunk.T @ [vh | 1] so column 128 accumulates
           the softmax denominator inside the same matmul; normalize on
           VectorE fused with the v-bias add, then transposed into attnT
           (ch, n) via DMA-xbar transpose (keeps TensorE free).
  phase 3: out(n,768) = attnT_chunk.T @ Wo chunks, fp32 out.
"""

import numpy as np
import ml_dtypes
from contextlib import ExitStack

import concourse.bass as bass
import concourse.mybir as mybir
import concourse.tile as tile
from concourse import bacc
from concourse.bass_utils import run_bass_kernel_spmd

B = 4
N_CTX = 4096
DM = 768
CH = 1024          # channels per core (8 heads x 128)
HD = 128
NH = 8             # heads per core
KC = DM // 128     # 6 contraction chunks for projections
NT = 512           # n-tile width
SCALE = HD ** -0.5
LOG2E = 1.4426950408889634
C_ZM = 7.219274    # zero-mean Schraudolph constant (2^7 mantissa domain)

f32 = mybir.dt.float32
bf16 = mybir.dt.bfloat16
i16 = mybir.dt.int16
fp8 = mybir.dt.float8e4
AF = mybir.ActivationFunctionType
ALU = mybir.AluOpType
PM = mybir.MatmulPerfMode
NTP = 1024         # phase-1 n-tile width (wide moving to amortize LDWEIGHTS)

_CACHE = {}


def _install_profhook():
    import contextlib, ctypes, sys, types

    if "antenv.axon_hooks" in sys.modules:
        return
    so = "/opt/axon/libaxon_pjrt.so"
    try:
        lib = ctypes.CDLL(so)
    except OSError:
        return
    if not hasattr(lib, "axon_start_nrt_profile"):
        return
    lib.axon_start_nrt_profile.argtypes = [ctypes.POINTER(ctypes.c_int64), ctypes.c_size_t]
    lib.axon_start_nrt_profile.restype = ctypes.c_int64
    lib.axon_stop_nrt_profile.argtypes = [ctypes.c_char_p]
    lib.axon_stop_nrt_profile.restype = ctypes.c_int64

    @contextlib.contextmanager
    def _hook(output_dir, device_ids):
        import jax
        jax.devices()
        if device_ids:
            ids = (ctypes.c_int64 * len(device_ids))(*device_ids)
            rc = lib.axon_start_nrt_profile(ids, len(device_ids))
        else:
            rc = lib.axon_start_nrt_profile(None, 0)
        if rc != 0:
            raise RuntimeError(f"axon_start_nrt_profile rc={rc}")
        try:
            yield
        finally:
            nf = lib.axon_stop_nrt_profile(str(output_dir).encode())
            print(f"profile: {nf} ntff file(s) in {output_dir}", file=sys.stderr)

    mod = types.ModuleType("antenv.axon_hooks")
    mod.get_axon_ntff_profile_hook = lambda: _hook
    mod.set_axon_ntff_profile_hook = lambda h: None
    sys.modules["antenv.axon_hooks"] = mod

    import concourse.bass_utils as bu
    bu.upload_artifacts = lambda tmpdir: "local://" + str(tmpdir)


def _build(n=N_CTX):
    nts = n // NT          # n tiles
    nbs = n // 128         # n blocks
    nc = bacc.Bacc(None, target_bir_lowering=False, debug=False, num_devices=8)

    qT = nc.declare_dram_parameter("qT", [DM, n], fp8, isOutput=False)
    kT = nc.declare_dram_parameter("kT", [DM, n], fp8, isOutput=False)
    vT = nc.declare_dram_parameter("vT", [DM, n], bf16, isOutput=False)
    wq = nc.declare_dram_parameter("wq", [DM, CH], fp8, isOutput=False)
    wk = nc.declare_dram_parameter("wk", [DM, CH], fp8, isOutput=False)
    wv = nc.declare_dram_parameter("wv", [DM, CH], bf16, isOutput=False)
    wo = nc.declare_dram_parameter("wo", [CH, DM], bf16, isOutput=False)
    bqk = nc.declare_dram_parameter("bqk", [128, 2 * NH], f32, isOutput=False)
    bvb = nc.declare_dram_parameter("bvb", [128, CH], f32, isOutput=False)
    out = nc.declare_dram_parameter("out", [n, DM], f32, isOutput=True)

    with tile.TileContext(nc) as tc, ExitStack() as ctx:
        dram = ctx.enter_context(tc.tile_pool(name="dram", bufs=1, space="DRAM"))
        qh_s = dram.tile([NH, 128, n], bf16)
        kh_s = dram.tile([NH, 128, n], bf16)
        vh_s = dram.tile([nbs, 128, NH, HD], bf16)

        singles = ctx.enter_context(tc.tile_pool(name="singles", bufs=1))

        bqk_t = singles.tile([128, 2 * NH], f32)
        nc.sync.dma_start(out=bqk_t[:, :], in_=bqk[:, :])
        bvb_t = singles.tile([128, CH], f32)
        nc.sync.dma_start(out=bvb_t[:, :], in_=bvb[:, :])

        # ---------------- phase 1a: q/k projections ----------------
        p1 = ExitStack()
        wpool = p1.enter_context(tc.tile_pool(name="w1", bufs=1))
        stream = p1.enter_context(tc.tile_pool(name="stream", bufs=3))
        stage1 = p1.enter_context(tc.tile_pool(name="stage1", bufs=4))
        pp = p1.enter_context(tc.tile_pool(name="pp", bufs=4, space="PSUM"))

        wq_t = wpool.tile([128, KC, CH], fp8, tag="wq")
        wk_t = wpool.tile([128, KC, CH], fp8, tag="wk")
        wv_t = wpool.tile([128, KC, CH], bf16, tag="wv")
        for w_t, w in ((wq_t, wq), (wk_t, wk), (wv_t, wv)):
            nc.sync.dma_start(
                out=w_t[:, :, :], in_=w[:].rearrange("(c p) m -> p c m", p=128)
            )

        ntsp = n // NTP
        for nt in range(ntsp):
            xs = []
            for name, x in (("q", qT), ("k", kT)):
                x_t = stream.tile([128, KC, NTP], fp8, tag=f"s{name}")
                nc.sync.dma_start(
                    out=x_t[:, :, :],
                    in_=x[:].rearrange("(c p) m -> p c m", p=128)[
                        :, :, nt * NTP : (nt + 1) * NTP
                    ],
                )
                xs.append(x_t)
            xv_t = stream.tile([128, KC, NTP], bf16, tag="sv")
            nc.sync.dma_start(
                out=xv_t[:, :, :],
                in_=vT[:].rearrange("(c p) m -> p c m", p=128)[
                    :, :, nt * NTP : (nt + 1) * NTP
                ],
            )
            for ti, (x_t, sc) in enumerate(((xs[0], qh_s), (xs[1], kh_s))):
                w_t = (wq_t, wk_t)[ti]
                for h in range(NH):
                    ps = pp.tile([128, NTP], f32, tag="proj")
                    # fp8 DoubleRow: contract chunk pair (2c, 2c+1); fp8
                    # moving operand caps at 1024 elems so go per 512-half
                    for half in range(NTP // 512):
                        hs = slice(half * 512, (half + 1) * 512)
                        for c in range(KC // 2):
                            nc.tensor.matmul(
                                ps[:, hs],
                                w_t[:, 2 * c : 2 * c + 2, h * HD : (h + 1) * HD],
                                x_t[:, 2 * c : 2 * c + 2, hs],
                                start=(c == 0),
                                stop=(c == KC // 2 - 1),
                                perf_mode=PM.DoubleRow,
                                skip_group_check=True,
                            )
                    st = stage1.tile([128, NTP], bf16, tag="qkst")
                    # bias add on ScalarE (idle during phase 1)
                    nc.scalar.activation(
                        st[:, :], ps[:, :], AF.Identity,
                        bias=bqk_t[:, ti * NH + h : ti * NH + h + 1],
                    )
                    nc.sync.dma_start(
                        out=sc[h, :, nt * NTP : (nt + 1) * NTP], in_=st[:, :]
                    )
            # v projection for this n-tile (no bias; folded post-PV)
            for nbq in range(NTP // 128):
                ps = pp.tile([128, CH], f32, tag="proj")
                for ct in range(CH // 512):
                    cs = slice(ct * 512, (ct + 1) * 512)
                    for c in range(KC):
                        nc.tensor.matmul(
                            ps[:, cs],
                            xv_t[:, c, nbq * 128 : (nbq + 1) * 128],
                            wv_t[:, c, cs],
                            start=(c == 0),
                            stop=(c == KC - 1),
                            skip_group_check=True,
                        )
                st = stage1.tile([128, CH], bf16, tag="vst")
                nc.vector.tensor_copy(st[:, :], ps[:, :])
                nc.sync.dma_start(
                    out=vh_s[nt * (NTP // 128) + nbq, :, :, :], in_=st[:, :],
                )

        p1.close()

        # ---------------- phase 2: attention per head ----------------
        atile = ctx.enter_context(tc.tile_pool(name="atile", bufs=NH))
        p2 = ExitStack()
        qk2 = p2.enter_context(tc.tile_pool(name="qk2", bufs=2))
        vh2 = p2.enter_context(tc.tile_pool(name="vh2", bufs=2))
        probs = p2.enter_context(tc.tile_pool(name="probs", bufs=7))
        stage2 = p2.enter_context(tc.tile_pool(name="stage2", bufs=4))
        small = p2.enter_context(tc.tile_pool(name="small", bufs=4))
        scp = p2.enter_context(tc.tile_pool(name="scp", bufs=2, space="PSUM"))
        pvp = p2.enter_context(tc.tile_pool(name="pvp", bufs=2, space="PSUM"))

        GG = 12  # key-chunks per buffered probs tile
        EG = 3   # key-chunks per exp op (3 psum banks)
        # Schraudolph constants: probs = exp(score*SCALE) as bf16 bits
        sch_s1 = SCALE * LOG2E * 128.0
        sch_s2 = 127.0 * 128.0 - C_ZM
        expctr = 0  # global exp-group counter for the 4:3 scalar:vector split
        at_ts = []
        for h in range(NH):
            qh_t = qk2.tile([128, n], bf16, tag="qh")
            nc.sync.dma_start(out=qh_t[:, :], in_=qh_s[h, :, :])
            kh_t = qk2.tile([128, n], bf16, tag="kh")
            nc.sync.dma_start(out=kh_t[:, :], in_=kh_s[h, :, :])
            vh_t = vh2.tile([128, nbs, HD + 1], bf16, tag="vh")
            nc.sync.dma_start(
                out=vh_t[:, :, 0:HD],
                in_=vh_s[:, :, h, :].rearrange("c p j -> p c j"),
            )
            nc.vector.memset(vh_t[:, :, HD], 1.0)

            at_t = atile.tile([128, n], bf16, tag="at")
            at_ts.append(at_t)

            prev_pq = None
            for qt in range(nts + 1):
              if qt < nts:
                pq_list = []
                for gq in range((nbs + GG - 1) // GG):
                    kb0 = gq * GG
                    kbn = min(GG, nbs - kb0)
                    pq = probs.tile([128, GG, NT], bf16, tag="pq")
                    pq_list.append(pq)
                    off = 0
                    while off < kbn:
                        gsz = min(EG, kbn - off)
                        sc_t = scp.tile([128, EG, NT], f32, tag="sc")
                        for i in range(gsz):
                            kb = kb0 + off + i
                            nc.tensor.matmul(
                                sc_t[:, i, :],
                                kh_t[:, kb * 128 : (kb + 1) * 128],
                                qh_t[:, qt * NT : (qt + 1) * NT],
                                start=True,
                                stop=True,
                            )
                        if expctr % 7 < 4:
                            nc.scalar.activation(
                                pq[:, off : off + gsz, :], sc_t[:, 0:gsz, :],
                                AF.Exp, scale=SCALE,
                            )
                        else:
                            nc.vector.tensor_scalar(
                                pq[:, off : off + gsz, :].bitcast(i16),
                                sc_t[:, 0:gsz, :],
                                sch_s1, sch_s2, ALU.mult, ALU.add,
                            )
                        expctr += 1
                        off += gsz
              if qt > 0:
                plist = prev_pq
                for qb in range(4):
                    pv = pvp.tile([128, HD + 1], f32, tag="pv")
                    for kb in range(nbs):
                        nc.tensor.matmul(
                            pv[:, :],
                            plist[kb // GG][:, kb % GG, qb * 128 : (qb + 1) * 128],
                            vh_t[:, kb, :],
                            start=(kb == 0),
                            stop=(kb == nbs - 1),
                            skip_group_check=True,
                        )
                    rec = small.tile([128, 1], f32, tag="rec")
                    nc.vector.reciprocal(rec[:, :], pv[:, HD : HD + 1])
                    st = stage2.tile([128, HD], bf16, tag="nst")
                    # st = pv/denom + bv  (one fused DVE op)
                    nc.vector.scalar_tensor_tensor(
                        st[:, :], pv[:, 0:HD], rec[:, :],
                        bvb_t[:, h * HD : (h + 1) * HD],
                        ALU.mult, ALU.add,
                    )
                    qb_g = (qt - 1) * 4 + qb
                    # transpose via DMA xbar (frees TensorE + avoids queue stall)
                    nc.sync.dma_start_transpose(
                        out=at_t[:, qb_g * 128 : (qb_g + 1) * 128], in_=st[:, :]
                    )
              if qt < nts:
                prev_pq = pq_list

        p2.close()

        # ---------------- phase 3: output projection ----------------
        wpool3 = ctx.enter_context(tc.tile_pool(name="w3", bufs=1))
        stage3 = ctx.enter_context(tc.tile_pool(name="stage3", bufs=4))
        opp = ctx.enter_context(tc.tile_pool(name="opp", bufs=2, space="PSUM"))
        wo_t = wpool3.tile([128, NH, DM], bf16, tag="wo")
        nc.sync.dma_start(
            out=wo_t[:, :, :], in_=wo[:].rearrange("(c p) m -> p c m", p=128)
        )
        for nb in range(nbs):
            po = opp.tile([128, DM], f32, tag="po")
            for h in range(NH):
                lhsT = at_ts[h][:, nb * 128 : (nb + 1) * 128]
                nc.tensor.matmul(
                    po[:, 0:512], lhsT, wo_t[:, h, 0:512],
                    start=(h == 0), stop=(h == NH - 1), skip_group_check=True,
                )
                nc.tensor.matmul(
                    po[:, 512:DM], lhsT, wo_t[:, h, 512:DM],
                    start=(h == 0), stop=(h == NH - 1), skip_group_check=True,
                )
            so = stage3.tile([128, DM], f32, tag="ost")
            nc.vector.tensor_copy(so[:, :], po[:, :])
            nc.sync.dma_start(out=out[nb * 128 : (nb + 1) * 128, :], in_=so[:, :])

    nc.compile()
    return nc


def _get_nc(n=N_CTX):
    if n not in _CACHE:
        _CACHE[n] = _build(n)
    return _CACHE[n]


def _shard_inputs(q, k, v, Wq, bq, Wk, bk, Wv, bv, Wo, bo):
    bf = ml_dtypes.bfloat16
    in_maps = []
    for c in range(8):
        bi, hg = c // 2, c % 2
        s = slice(hg * CH, (hg + 1) * CH)
        bqk_c = np.empty((128, 2 * NH), np.float32)
        for h in range(NH):
            bqk_c[:, h] = bq[hg * CH + h * HD : hg * CH + (h + 1) * HD]
            bqk_c[:, NH + h] = bk[hg * CH + h * HD : hg * CH + (h + 1) * HD]
        bvb_c = np.ascontiguousarray(
            np.broadcast_to(bv[s].astype(np.float32), (128, CH))
        )
        f8 = ml_dtypes.float8_e4m3fn
        in_maps.append({
            "qT": np.ascontiguousarray(q[bi].T).astype(f8),
            "kT": np.ascontiguousarray(k[bi].T).astype(f8),
            "vT": np.ascontiguousarray(v[bi].T).astype(bf),
            "wq": np.ascontiguousarray(Wq[:, s]).astype(f8),
            "wk": np.ascontiguousarray(Wk[:, s]).astype(f8),
            "wv": np.ascontiguousarray(Wv[:, s]).astype(bf),
            "wo": np.ascontiguousarray(Wo[s, :]).astype(bf),
            "bqk": bqk_c,
            "bvb": bvb_c,
        })
    return in_maps


def kernel(q, k, v, Wq, bq, Wk, bk, Wv, bv, Wo, bo, _profile=False):
    import os

    q = np.asarray(q); k = np.asarray(k); v = np.asarray(v)
    n = q.shape[1]
    nc = _get_nc(n)
    in_maps = _shard_inputs(
        q, k, v, np.asarray(Wq), np.asarray(bq), np.asarray(Wk), np.asarray(bk),
        np.asarray(Wv), np.asarray(bv), np.asarray(Wo), np.asarray(bo),
    )
    profile = _profile or bool(int(os.environ.get("KERNEL_PROFILE", "0")))
    if profile:
        _install_profhook()
    res = run_bass_kernel_spmd(nc, in_maps, list(range(8)), trace=profile)
    if profile and res.exec_time_ns is not None:
        print(f"HW exec time: {res.exec_time_ns} ns")
    bo32 = np.asarray(bo, np.float32)
    out = np.empty((q.shape[0], n, DM), np.float32)
    for bi in range(q.shape[0]):
        out[bi] = res.results[2 * bi]["out"] + res.results[2 * bi + 1]["out"] + bo32
    return out
